# revision 6
# baseline (speedup 1.0000x reference)
"""DeepSeek MoE router kernel for 8x TRN2 NeuronCores (Bass/Tile).

Computes, for x[4,8192,2048], gate_w[64,2048], expert_bias[64], noise[32768,64]:
  logits = x @ gate_w.T + noise*0.01 + expert_bias
  probs = softmax(logits); top8 weights (renormalized) + indices;
  per-expert counts -> load-balance bias update.

Sharding: tokens (32768) split 8 ways; gate_w/expert_bias replicated;
counts summed on host (64 floats); bias update on host (64 floats).

GEMM path: x tiles are PE-transposed (fp32) into PSUM, copied to SBUF with
FP32R rounding (ACT/DVE alternating per k-chunk), then a single-pass FP32R
matmul accumulates logitsT[64, 512] over 16 k-chunks. FP32R keeps ~full fp32
mantissa relevant here and streams at 1 cycle/row (vs 4 for plain fp32).

The toolchain allows exactly ONE embedded sync-wait per TPB instruction, so
the instruction graph is arranged so every op has at most one un-observed
cross-engine dependency (same-semaphore deps merge; older values are covered
by each engine's monotonic observations). Tiny absorber ops pre-observe DMA
semaphores where an op would otherwise need two waits.
"""
import sys

sys.path.insert(0, "/opt/trn_rl_repo")

import numpy as np
import ml_dtypes

import concourse.bass as bass
import concourse.tile as tile
from concourse import bacc
from concourse import mybir
from concourse import bass_utils

F32 = mybir.dt.float32
F32R = mybir.dt.float32r
BF16 = mybir.dt.bfloat16
U32 = mybir.dt.uint32

HIDDEN = 2048
NUM_EXPERTS = 64
TOP_K = 8
JITTER = 0.01
BIAS_UPDATE_RATE = 0.001
T_TOTAL = 32768
N_CORES = 8
T_CORE = T_TOTAL // N_CORES          # 4096 tokens per core
BLK = 512                            # tokens per block
N_BLK = T_CORE // BLK                # 8 blocks
KCH = HIDDEN // 128                  # 16 k-chunks

_CACHE = {}


def _build_nc():
    nc = bacc.Bacc("TRN2", target_bir_lowering=False, debug=False)

    X = nc.dram_tensor("X", [T_CORE, HIDDEN], F32, kind="ExternalInput").ap()
    NZ = nc.dram_tensor("NZ", [T_CORE, NUM_EXPERTS], F32, kind="ExternalInput").ap()
    WT = nc.dram_tensor("WT", [HIDDEN, NUM_EXPERTS], F32, kind="ExternalInput").ap()
    BIASB = nc.dram_tensor("BIASB", [128, 4 * NUM_EXPERTS], F32, kind="ExternalInput").ap()
    IDEN = nc.dram_tensor("IDEN", [128, 128], F32, kind="ExternalInput").ap()
    ONES = nc.dram_tensor("ONES", [128, 1], BF16, kind="ExternalInput").ap()

    PROBS = nc.dram_tensor("PROBS", [T_CORE, NUM_EXPERTS], F32, kind="ExternalOutput").ap()
    W8 = nc.dram_tensor("W8", [T_CORE, TOP_K], F32, kind="ExternalOutput").ap()
    I8 = nc.dram_tensor("I8", [T_CORE, TOP_K], U32, kind="ExternalOutput").ap()
    COUNTS = nc.dram_tensor("COUNTS", [1, NUM_EXPERTS], F32, kind="ExternalOutput").ap()

    with tile.TileContext(nc) as tc:
        with (
            tc.tile_pool(name="const", bufs=1) as constp,
            tc.tile_pool(name="xin", bufs=8) as xin,
            tc.tile_pool(name="nzp", bufs=2) as nzp,
            tc.tile_pool(name="xtr_a", bufs=2) as xtr_a,
            tc.tile_pool(name="xtr_d", bufs=2) as xtr_d,
            tc.tile_pool(name="lts", bufs=2) as ltsp,
            tc.tile_pool(name="lsb", bufs=2) as lsbp,
            tc.tile_pool(name="esb", bufs=2) as esbp,
            tc.tile_pool(name="psb", bufs=N_BLK) as psbp,
            tc.tile_pool(name="small", bufs=N_BLK) as smallp,
            tc.tile_pool(name="ps_tp", bufs=2, space="PSUM") as ps_tp,
            tc.tile_pool(name="ps_lt", bufs=2, space="PSUM") as ps_lt,
            tc.tile_pool(name="ps_bt", bufs=2, space="PSUM") as ps_bt,
            tc.tile_pool(name="ps_ct", bufs=1, space="PSUM") as ps_ct,
        ):
            # ---- preamble: constants -------------------------------------
            iden = constp.tile([128, 128], F32)
            nc.sync.dma_start(iden[:], IDEN)
            ones = constp.tile([128, 1], BF16)
            nc.sync.dma_start(ones[:], ONES)
            wt_sb = constp.tile([128, KCH, NUM_EXPERTS], F32)
            nc.sync.dma_start(
                wt_sb[:], WT.rearrange("(c p) e -> p c e", p=128)
            )
            biasb = constp.tile([128, 4 * NUM_EXPERTS], F32)
            nc.sync.dma_start(biasb[:], BIASB)

            # PE absorbers: observe IDEN and ONES DMA sems
            pre_tp = ps_tp.tile([128, BLK], F32, tag="tp", name="pre_tp")
            nc.tensor.transpose(pre_tp[:, 0:128], iden[:], iden[:])
            pre_cm = ps_ct.tile([1, 1], F32, tag="precm", name="pre_cm")
            nc.tensor.matmul(pre_cm[:], ones[:], ones[:, 0:1], start=True, stop=True)

            # ACT: wT chunks rounded to f32r (absorbs WT DMA sem)
            wtr = constp.tile([128, KCH, NUM_EXPERTS], F32R)
            for kc in range(KCH):
                nc.scalar.activation(
                    wtr[:, kc, :], wt_sb[:, kc, :],
                    mybir.ActivationFunctionType.Copy,
                )

            counts_ps = ps_ct.tile([1, NUM_EXPERTS], F32)
            dve_scr = constp.tile([1, 1], F32)

            for b in range(N_BLK):
                # ---- load x tiles + noise --------------------------------
                x_sb = [
                    xin.tile([128, HIDDEN], F32, tag="xsb", name=f"xsb_{b}_{t}")
                    for t in range(4)
                ]
                for t in range(4):
                    nc.sync.dma_start(
                        x_sb[t][:], X[b * BLK + t * 128 : b * BLK + (t + 1) * 128, :]
                    )
                nz_sb = nzp.tile([128, 4, NUM_EXPERTS], F32, tag="nz")
                nc.sync.dma_start(
                    nz_sb[:],
                    NZ[b * BLK : (b + 1) * BLK, :].rearrange("(t p) e -> p t e", p=128),
                )

                # ---- GEMM: logitsT[64, 512] over 16 k-chunks -------------
                lt_ps = ps_lt.tile([NUM_EXPERTS, BLK], F32, tag="lt")
                for kc in range(KCH):
                    tp_ps = ps_tp.tile([128, BLK], F32, tag="tp")
                    for t in range(4):
                        nc.tensor.transpose(
                            tp_ps[:, t * 128 : (t + 1) * 128],
                            x_sb[t][:, kc * 128 : (kc + 1) * 128],
                            iden[:],
                        )
                    pool = xtr_a if kc % 2 == 0 else xtr_d
                    xtr = pool.tile([128, BLK], F32R, tag="xtr")
                    if kc % 2 == 0:
                        nc.scalar.activation(
                            xtr[:], tp_ps[:], mybir.ActivationFunctionType.Copy
                        )
                    else:
                        nc.vector.tensor_copy(xtr[:], tp_ps[:])
                    nc.tensor.matmul(
                        lt_ps[:], wtr[:, kc, :], xtr[:],
                        start=(kc == 0), stop=(kc == KCH - 1),
                    )

                # ---- logitsT -> SBUF (ACT), back-transpose (PE) ----------
                lt_sb = ltsp.tile([NUM_EXPERTS, BLK], F32, tag="ltsb")
                nc.scalar.activation(
                    lt_sb[:], lt_ps[:], mybir.ActivationFunctionType.Copy
                )
                bt_ps = ps_bt.tile([128, 4 * NUM_EXPERTS], F32, tag="bt")
                for t in range(4):
                    nc.tensor.transpose(
                        bt_ps[:, t * NUM_EXPERTS : (t + 1) * NUM_EXPERTS],
                        lt_sb[:, t * 128 : (t + 1) * 128],
                        iden[0:NUM_EXPERTS, 0:NUM_EXPERTS],
                    )

                # ---- logits = psum + noise*J, + bias (DVE) ---------------
                # absorber: observe this block's noise DMA on DVE
                nc.vector.tensor_copy(dve_scr[:], nz_sb[0:1, 0, 0:1])
                l_sb = lsbp.tile([128, 4, NUM_EXPERTS], F32, tag="lsb")
                nc.vector.tensor_add(
                    l_sb[:],
                    bt_ps[:].rearrange("p (t e) -> p t e", e=NUM_EXPERTS),
                    nz_sb[:],
                )
                nc.vector.tensor_add(
                    l_sb[:], l_sb[:],
                    biasb[:].rearrange("p (t e) -> p t e", e=NUM_EXPERTS),
                )

                # ---- softmax + top8 --------------------------------------
                nmax = smallp.tile([128, 4], F32, tag="nmax")
                for t in range(4):
                    nc.vector.tensor_reduce(
                        out=nmax[:, t : t + 1], in_=l_sb[:, t, :],
                        op=mybir.AluOpType.max, axis=mybir.AxisListType.X,
                        negate=True,
                    )
                e_sb = esbp.tile([128, 4, NUM_EXPERTS], F32, tag="esb")
                sume = smallp.tile([128, 4], F32, tag="sume")
                for t in range(4):
                    nc.scalar.activation(
                        e_sb[:, t, :], l_sb[:, t, :],
                        mybir.ActivationFunctionType.Exp,
                        bias=nmax[:, t : t + 1], scale=1.0,
                        accum_out=sume[:, t : t + 1],
                    )
                rsum = smallp.tile([128, 4], F32, tag="rsum")
                nc.vector.reciprocal(rsum[:], sume[:])
                p_sb = psbp.tile([128, 4, NUM_EXPERTS], F32, tag="psb")
                for t in range(4):
                    nc.scalar.activation(
                        p_sb[:, t, :], e_sb[:, t, :],
                        mybir.ActivationFunctionType.Copy,
                        scale=rsum[:, t : t + 1],
                    )

                top8 = smallp.tile([128, 4, TOP_K], F32, tag="top8")
                idx8 = smallp.tile([128, 4, TOP_K], U32, tag="idx8")
                for t in range(4):
                    nc.vector.max(top8[:, t, :], p_sb[:, t, :])
                for t in range(4):
                    nc.vector.max_index(idx8[:, t, :], top8[:, t, :], p_sb[:, t, :])

                s8 = smallp.tile([128, 4], F32, tag="s8")
                for t in range(4):
                    nc.vector.tensor_reduce(
                        out=s8[:, t : t + 1], in_=top8[:, t, :],
                        op=mybir.AluOpType.add, axis=mybir.AxisListType.X,
                    )
                r8 = smallp.tile([128, 4], F32, tag="r8")
                nc.vector.reciprocal(r8[:], s8[:])
                w8_sb = smallp.tile([128, 4, TOP_K], F32, tag="w8")
                for t in range(4):
                    nc.vector.tensor_scalar_mul(
                        w8_sb[:, t, :], top8[:, t, :], r8[:, t : t + 1]
                    )

                # ---- counts: mask = probs >= kth value; PE accumulates ---
                mask = smallp.tile([128, 4, NUM_EXPERTS], BF16, tag="mask")
                for t in range(4):
                    nc.vector.tensor_scalar(
                        out=mask[:, t, :], in0=p_sb[:, t, :],
                        scalar1=top8[:, t, TOP_K - 1 : TOP_K], scalar2=None,
                        op0=mybir.AluOpType.is_ge,
                    )
                for t in range(4):
                    nc.tensor.matmul(
                        counts_ps[:], ones[:], mask[:, t, :],
                        start=(b == 0 and t == 0),
                        stop=(b == N_BLK - 1 and t == 3),
                    )

                # ---- outputs ---------------------------------------------
                nc.sync.dma_start(
                    PROBS[b * BLK : (b + 1) * BLK, :].rearrange(
                        "(t p) e -> p t e", p=128
                    ),
                    p_sb[:],
                )
                nc.sync.dma_start(
                    W8[b * BLK : (b + 1) * BLK, :].rearrange(
                        "(t p) e -> p t e", p=128
                    ),
                    w8_sb[:],
                )
                nc.sync.dma_start(
                    I8[b * BLK : (b + 1) * BLK, :].rearrange(
                        "(t p) e -> p t e", p=128
                    ),
                    idx8[:],
                )

            counts_sb = constp.tile([1, NUM_EXPERTS], F32)
            nc.vector.tensor_copy(counts_sb[:], counts_ps[:])
            nc.sync.dma_start(COUNTS, counts_sb[:])

    nc.compile()
    return nc


def kernel(x, gate_w, expert_bias, noise):
    x = np.asarray(x, dtype=np.float32)
    gate_w = np.asarray(gate_w, dtype=np.float32)
    expert_bias = np.asarray(expert_bias, dtype=np.float32)
    noise = np.asarray(noise, dtype=np.float32)

    if "nc" not in _CACHE:
        _CACHE["nc"] = _build_nc()
    nc = _CACHE["nc"]

    flat_x = np.ascontiguousarray(x.reshape(T_TOTAL, HIDDEN))
    nz = noise * np.float32(JITTER)
    wt = np.ascontiguousarray(gate_w.T)
    biasb = np.ascontiguousarray(
        np.tile(expert_bias[None, :], (128, 4)).astype(np.float32)
    )
    iden = np.eye(128, dtype=np.float32)
    ones = np.ones((128, 1), dtype=ml_dtypes.bfloat16)

    in_maps = []
    for c in range(N_CORES):
        sl = slice(c * T_CORE, (c + 1) * T_CORE)
        in_maps.append(
            {
                "X": np.ascontiguousarray(flat_x[sl]),
                "NZ": np.ascontiguousarray(nz[sl]),
                "WT": wt,
                "BIASB": biasb,
                "IDEN": iden,
                "ONES": ones,
            }
        )

    _CACHE["last_in_maps"] = in_maps
    res = bass_utils.run_bass_kernel_spmd(nc, in_maps, core_ids=list(range(N_CORES)))
    outs = res.results

    probs = np.concatenate([o["PROBS"] for o in outs], axis=0)
    weights = np.concatenate([o["W8"] for o in outs], axis=0)
    indices = np.concatenate([o["I8"] for o in outs], axis=0).view(np.int32)
    counts = np.sum([o["COUNTS"][0] for o in outs], axis=0, dtype=np.float32)

    load = counts / np.float32(T_TOTAL * TOP_K)
    error = load - np.float32(1.0 / NUM_EXPERTS)
    new_bias = expert_bias - np.float32(BIAS_UPDATE_RATE) * np.sign(error, dtype=np.float32)

    return weights, indices, probs, new_bias
